# revision 6
# baseline (speedup 1.0000x reference)
"""DecodeBox (nms_detection) Trainium2 Bass kernel, 8-core data-parallel, fp16 I/O.

v9 = v5/v7 design with VARIABLE tile sizes [512,512,512,512,768,256]: the
exec window ends at (last big-tanh end + DVE's last-tile workload), so a
small final tile shrinks the coda while tile 4 absorbs the difference --
total ACT elements and op count (and so ACT busy) are unchanged.

See kernel.py history for the measured design rules: fp16 HBM I/O both ways,
f32 tanh scratch (f16 cancels), unit-stride engine writes only (strided
2-byte writes are 2.2-2.4x slow), host does the [pos,attr] interleave, all
DMAs fully contiguous on the sync HWDGE ring (first byte ~2.9us fixed
kickoff), dummy 1-elem Tanh preloads the ACT table, in0 lands in two pieces.
"""

import numpy as np

B, A, ATTRS = 4, 3, 10
D = H = W = 64
S = D * H * W              # 262144 positions per (b, a) slab
SH = S // 2                # 131072 positions per half-slab
NCORES = 8
HS_PER_CORE = 3            # 24 half-slabs / 8 cores
P = 128                    # SBUF partitions
R = SH // P                # 1024 positions per partition per half-slab
TILES = [512, 512, 512, 512, 640, 384]   # per-tile positions/partition
TILE_HS = [0, 0, 1, 1, 2, 2]             # half-slab of each tile
TILE_OFF = [0, 512, 0, 512, 0, 640]      # column offset within the half-slab
NT = len(TILES)
CUM = np.concatenate([[0], np.cumsum([ATTRS * f for f in TILES])]).tolist()
NSCR = 3                   # f32 tanh-scratch ring depth (slot k serves tiles k, k+3)
SCR_F = [max(TILES[k], TILES[k + 3]) for k in range(NSCR)]
SPLIT0 = 4 * TILES[0]      # in0 lands as attr rows 0-3, then rows 4-9
ANCHOR_W = np.array([10.0, 16.0, 33.0], dtype=np.float32)
# const layout (columns of [P, NCONST]): gxrow(64) | gysm(16) | gzb(3) | lnanc(3)
NGY = R // W               # 16 gysm rows covering a full half-slab
NCONST = W + NGY + HS_PER_CORE + HS_PER_CORE

_CACHE = {}


def _build_nc():
    import contextlib

    import concourse.bass as bass
    import concourse.mybir as mybir

    AFT = mybir.ActivationFunctionType
    add = mybir.AluOpType.add
    mult = mybir.AluOpType.mult
    f32 = mybir.dt.float32
    f16 = mybir.dt.float16

    nc = bass.Bass()
    # Flat tile-major DRAM layout: each tile's [P, 10F] slab is one fully
    # contiguous DRAM region, so every DMA is a single maximal descriptor.
    xin = nc.dram_tensor("xin", [1, P * CUM[NT]], f16, kind="ExternalInput")
    consts = nc.dram_tensor("consts", [P, NCONST], f32, kind="ExternalInput")
    yout = nc.dram_tensor("yout", [1, P * CUM[NT]], f16, kind="ExternalOutput")

    with contextlib.ExitStack() as stack:
        ctile = stack.enter_context(nc.sbuf_tensor("ctile", [P, NCONST], f32))
        in_t = [
            stack.enter_context(nc.sbuf_tensor(f"in{i}", [P, ATTRS * TILES[i]], f16))
            for i in range(NT)
        ]
        # f32 tanh scratch: lanes 0-2 at [0,3F), lanes 4-9 at [3F,9F)
        t_t = [
            stack.enter_context(nc.sbuf_tensor(f"t{k}", [P, 9 * SCR_F[k]], f32))
            for k in range(NSCR)
        ]
        out_t = [
            stack.enter_context(nc.sbuf_tensor(f"out{i}", [P, ATTRS * TILES[i]], f16))
            for i in range(NT)
        ]
        const_done = stack.enter_context(nc.semaphore("const_done"))
        in_done = stack.enter_context(nc.semaphore("in_done"))
        out_done = stack.enter_context(nc.semaphore("out_done"))  # DGE sync info
        act_done = stack.enter_context(nc.semaphore("act_done"))
        dve_done = stack.enter_context(nc.semaphore("dve_done"))
        block = stack.enter_context(nc.Block())

        o = 0
        gxrow = ctile[:, o:o + W]; o += W            # 2 + 4*j0   [P, 64]
        gysm = ctile[:, o:o + NGY]; o += NGY         # [P, 16]
        gzb = ctile[:, o:o + HS_PER_CORE]; o += HS_PER_CORE   # z-lane bias
        lnanc = ctile[:, o:o + HS_PER_CORE]                   # ln(anchor_w[a])

        @block.gpsimd
        def _(gpsimd):
            gpsimd.dma_start(out=ctile[:, :], in_=consts[:, :]).then_inc(const_done, 16)

        @block.sync
        def _(sync):
            c0 = ATTRS * TILES[0]
            xin0 = xin[:, :P * c0].rearrange("x (p c) -> p (x c)", c=c0)
            sync.dma_start(
                out=in_t[0][:, :SPLIT0], in_=xin0[:, :SPLIT0]
            ).then_inc(in_done, 16)
            sync.dma_start(
                out=in_t[0][:, SPLIT0:], in_=xin0[:, SPLIT0:]
            ).then_inc(in_done, 16)
            for i in range(1, NT):
                ci = ATTRS * TILES[i]
                sync.dma_start(
                    out=in_t[i][:, :],
                    in_=xin[:, P * CUM[i]:P * CUM[i + 1]].rearrange(
                        "x (p c) -> p (x c)", c=ci
                    ),
                ).then_inc(in_done, 16)
            for k in range(NT):
                ck = ATTRS * TILES[k]
                sync.wait_ge(dve_done, k + 1)
                sync.wait_ge(act_done, 3 * k + 3)  # exp lane written by ACT
                sync.dma_start(
                    out=yout[:, P * CUM[k]:P * CUM[k + 1]].rearrange(
                        "x (p c) -> p (x c)", c=ck
                    ),
                    in_=out_t[k][:, :],
                ).then_inc(out_done, 16)

        @block.scalar
        def _(scalar):
            # 1-element dummy triggers the ~1.3 us ACT_TABLE_LOAD under in0.
            nc.scalar.activation(t_t[0][:, 0:1], out_t[0][:, 0:1], AFT.Tanh)
            for i in range(NT):
                F = TILES[i]
                hs = TILE_HS[i]
                scalar.wait_ge(in_done, 16 * (i + 2) if i else 16)
                if i == 0:
                    scalar.wait_ge(const_done, 16)  # lnanc for the exp bias
                if i >= NSCR:
                    scalar.wait_ge(dve_done, i - NSCR + 1)  # t-scratch reuse
                in_r = in_t[i].rearrange("p (a j) -> p a j", a=ATTRS)
                t_r = t_t[i % NSCR].rearrange("p (a j) -> p a j", a=9)[:, :, :F]
                out_r = out_t[i].rearrange("p (a j) -> p a j", a=ATTRS)
                op_xyz = lambda: nc.scalar.activation(
                    t_r[:, 0:3, :], in_r[:, 0:3, :], AFT.Tanh, scale=0.5
                ).then_inc(act_done, 1)
                op_exp = lambda: nc.scalar.activation(
                    out_r[:, 3:4, :], in_r[:, 3:4, :], AFT.Exp,
                    bias=lnanc[:, hs:hs + 1],
                ).then_inc(act_done, 1)
                op_cls = lambda: nc.scalar.activation(
                    t_r[:, 3:9, :], in_r[:, 4:10, :], AFT.Tanh, scale=0.5
                ).then_inc(act_done, 1)
                if i == 0:
                    op_xyz(); op_exp()
                    scalar.wait_ge(in_done, 32)  # rows 4-9 of in0
                    op_cls()
                else:
                    op_cls(); op_xyz(); op_exp()

        @block.vector
        def _(vector):
            vector.wait_ge(const_done, 16)
            for i in range(NT):
                F = TILES[i]
                F1 = F // W
                hs = TILE_HS[i]
                g0 = TILE_OFF[i] // W
                t_r = t_t[i % NSCR].rearrange("p (a j) -> p a j", a=9)[:, :, :F]
                t_r4 = t_t[i % NSCR].rearrange(
                    "p (a j1 j0) -> p a j1 j0", a=9, j0=W
                )[:, :, :F1, :]
                out_r = out_t[i].rearrange("p (a j) -> p a j", a=ATTRS)
                out_r4 = out_t[i].rearrange(
                    "p (a j1 j0) -> p a j1 j0", a=ATTRS, j0=W
                )
                gx_bc = gxrow.unsqueeze(1).broadcast_to([P, F1, W])
                gy_bc = gysm[:, g0:g0 + F1].unsqueeze(2).broadcast_to([P, F1, W])
                op_big = lambda: nc.vector.tensor_scalar(
                    out_r[:, 4:10, :], t_r[:, 3:9, :], 0.5, 0.5, mult, add
                )
                op_x = lambda: nc.vector.scalar_tensor_tensor(
                    out_r4[:, 0], t_r4[:, 0], 2.0, gx_bc, mult, add
                )
                op_y = lambda: nc.vector.scalar_tensor_tensor(
                    out_r4[:, 1], t_r4[:, 1], 2.0, gy_bc, mult, add
                )
                op_z = lambda: nc.vector.tensor_scalar(
                    out_r[:, 2, :], t_r[:, 2, :], 2.0, gzb[:, hs:hs + 1], mult, add
                )
                if i == 0:
                    vector.wait_ge(act_done, 1)
                    op_x(); op_y(); op_z()
                    vector.wait_ge(act_done, 3)
                    op_big().then_inc(dve_done, 1)
                else:
                    vector.wait_ge(act_done, 3 * i + 1)
                    op_big()
                    vector.wait_ge(act_done, 3 * i + 2)
                    op_x(); op_y()
                    op_z().then_inc(dve_done, 1)

    return nc


def _host_constants():
    """Half-slab position s = p*R + off + j1*64 + j0:
      w = j0;  hgrid = 16*(p%4) + (off//64 + j1);  d = half*32 + p//4
    """
    p = np.arange(P)
    gxrow = np.broadcast_to(2.0 + 4.0 * np.arange(W), (P, W))
    rows = np.arange(NGY)  # off//64 + j1 over a full half-slab
    gysm = 2.0 + 4.0 * (16.0 * (p[:, None] % 4) + rows[None, :])
    base = np.concatenate([gxrow, gysm], axis=1)
    out = []
    for core in range(NCORES):
        gzb = np.empty((P, HS_PER_CORE), np.float32)
        lnanc = np.empty((P, HS_PER_CORE), np.float32)
        for k in range(HS_PER_CORE):
            hs_g = HS_PER_CORE * core + k
            slab, half = divmod(hs_g, 2)
            gzb[:, k] = 2.0 + 128.0 * half + 4.0 * (p // 4)
            lnanc[:, k] = np.log(ANCHOR_W[slab % A])
        out.append(np.concatenate([base, gzb, lnanc], axis=1).astype(np.float32))
    return out


def _run(inputs, trace=False):
    from concourse.bass_utils import run_bass_kernel_spmd

    x = np.asarray(inputs["input"])
    assert x.shape == (B, A * ATTRS, D, H, W), x.shape
    # -> f16 [hs_g, p, a, j(=R)] then concat per-tile [p, a, off:off+F]
    # column blocks so every load DMA is a straight contiguous memcpy.
    xh = np.ascontiguousarray(
        x.reshape(B * A, ATTRS, 2, P, R).transpose(0, 2, 3, 1, 4),
        dtype=np.float16,
    )  # [24, P, ATTRS, R] after merging slab+half
    xh = xh.reshape(B * A * 2, P, ATTRS, R)

    if "nc" not in _CACHE:
        _CACHE["nc"] = _build_nc()
        _CACHE["consts"] = _host_constants()
    nc = _CACHE["nc"]
    consts = _CACHE["consts"]

    in_maps = []
    for core in range(NCORES):
        pieces = []
        for i in range(NT):
            hs_g = HS_PER_CORE * core + TILE_HS[i]
            off, F = TILE_OFF[i], TILES[i]
            pieces.append(xh[hs_g, :, :, off:off + F].reshape(P, ATTRS * F))
        flat = np.concatenate([p.reshape(1, -1) for p in pieces], axis=1)
        in_maps.append({"xin": flat, "consts": consts[core]})

    res = run_bass_kernel_spmd(
        nc, in_maps, core_ids=list(range(NCORES)), trace=trace
    )
    _CACHE["last_exec_ns"] = res.exec_time_ns
    _CACHE["last_results"] = res

    # reassemble [hs_g, p, a, R] then -> [hs_g, p, j, a] -> [B, A*S, ATTRS]
    yh = np.empty((NCORES * HS_PER_CORE, P, ATTRS, R), np.float16)
    for core in range(NCORES):
        yc = res.results[core]["yout"]
        for i in range(NT):
            hs_g = HS_PER_CORE * core + TILE_HS[i]
            off, F = TILE_OFF[i], TILES[i]
            yh[hs_g, :, :, off:off + F] = yc.reshape(-1)[
                P * CUM[i]:P * CUM[i + 1]
            ].reshape(P, ATTRS, F)
    y = np.ascontiguousarray(yh.transpose(0, 1, 3, 2))
    return y.reshape(B, A * S, ATTRS).astype(np.float32)


def kernel(**inputs):
    return _run(inputs, trace=False)



# revision 7
# speedup vs baseline: 1.0106x; 1.0106x over previous
"""DecodeBox (nms_detection) Trainium2 Bass kernel, 8-core data-parallel, fp16 I/O.

v9 = v5/v7 design with VARIABLE tile sizes [512,512,512,512,768,256]: the
exec window ends at (last big-tanh end + DVE's last-tile workload), so a
small final tile shrinks the coda while tile 4 absorbs the difference --
total ACT elements and op count (and so ACT busy) are unchanged.

See kernel.py history for the measured design rules: fp16 HBM I/O both ways,
f32 tanh scratch (f16 cancels), unit-stride engine writes only (strided
2-byte writes are 2.2-2.4x slow), host does the [pos,attr] interleave, all
DMAs fully contiguous on the sync HWDGE ring (first byte ~2.9us fixed
kickoff), dummy 1-elem Tanh preloads the ACT table, in0 lands in two pieces.
"""

import numpy as np

B, A, ATTRS = 4, 3, 10
D = H = W = 64
S = D * H * W              # 262144 positions per (b, a) slab
SH = S // 2                # 131072 positions per half-slab
NCORES = 8
HS_PER_CORE = 3            # 24 half-slabs / 8 cores
P = 128                    # SBUF partitions
R = SH // P                # 1024 positions per partition per half-slab
TILES = [512, 512, 512, 512, 640, 384]   # per-tile positions/partition
TILE_HS = [0, 0, 1, 1, 2, 2]             # half-slab of each tile
TILE_OFF = [0, 512, 0, 512, 0, 640]      # column offset within the half-slab
NT = len(TILES)
CUM = np.concatenate([[0], np.cumsum([ATTRS * f for f in TILES])]).tolist()
NSCR = 3                   # f32 tanh-scratch ring depth (slot k serves tiles k, k+3)
SCR_F = [max(TILES[k], TILES[k + 3]) for k in range(NSCR)]
SPLIT0 = 4 * TILES[0]      # in0 lands as attr rows 0-3, then rows 4-9
ANCHOR_W = np.array([10.0, 16.0, 33.0], dtype=np.float32)
# const layout (columns of [P, NCONST]): gxrow(64) | gysm(16) | gzb(3) | lnanc(3)
NGY = R // W               # 16 gysm rows covering a full half-slab
NCONST = W + NGY + HS_PER_CORE + HS_PER_CORE

_CACHE = {}


def _build_nc():
    import contextlib

    import concourse.bass as bass
    import concourse.mybir as mybir

    AFT = mybir.ActivationFunctionType
    add = mybir.AluOpType.add
    mult = mybir.AluOpType.mult
    f32 = mybir.dt.float32
    f16 = mybir.dt.float16

    nc = bass.Bass()
    # Flat tile-major DRAM layout: each tile's [P, 10F] slab is one fully
    # contiguous DRAM region, so every DMA is a single maximal descriptor.
    xin = nc.dram_tensor("xin", [1, P * CUM[NT]], f16, kind="ExternalInput")
    consts = nc.dram_tensor("consts", [P, NCONST], f32, kind="ExternalInput")
    yout = nc.dram_tensor("yout", [1, P * CUM[NT]], f16, kind="ExternalOutput")

    with contextlib.ExitStack() as stack:
        ctile = stack.enter_context(nc.sbuf_tensor("ctile", [P, NCONST], f32))
        in_t = [
            stack.enter_context(nc.sbuf_tensor(f"in{i}", [P, ATTRS * TILES[i]], f16))
            for i in range(NT)
        ]
        # f32 tanh scratch: lanes 0-2 at [0,3F), lanes 4-9 at [3F,9F)
        t_t = [
            stack.enter_context(nc.sbuf_tensor(f"t{k}", [P, 9 * SCR_F[k]], f32))
            for k in range(NSCR)
        ]
        out_t = [
            stack.enter_context(nc.sbuf_tensor(f"out{i}", [P, ATTRS * TILES[i]], f16))
            for i in range(NT)
        ]
        const_done = stack.enter_context(nc.semaphore("const_done"))
        in_done = stack.enter_context(nc.semaphore("in_done"))
        out_done = stack.enter_context(nc.semaphore("out_done"))  # DGE sync info
        act_done = stack.enter_context(nc.semaphore("act_done"))
        dve_done = stack.enter_context(nc.semaphore("dve_done"))
        block = stack.enter_context(nc.Block())

        o = 0
        gxrow = ctile[:, o:o + W]; o += W            # 2 + 4*j0   [P, 64]
        gysm = ctile[:, o:o + NGY]; o += NGY         # [P, 16]
        gzb = ctile[:, o:o + HS_PER_CORE]; o += HS_PER_CORE   # z-lane bias
        lnanc = ctile[:, o:o + HS_PER_CORE]                   # ln(anchor_w[a])

        @block.gpsimd
        def _(gpsimd):
            gpsimd.dma_start(out=ctile[:, :], in_=consts[:, :]).then_inc(const_done, 16)

        @block.sync
        def _(sync):
            c0 = ATTRS * TILES[0]
            sync.dma_start(
                out=in_t[0][:, :SPLIT0],
                in_=xin[:, :P * SPLIT0].rearrange(
                    "x (p c) -> p (x c)", c=SPLIT0
                ),
            ).then_inc(in_done, 16)
            sync.dma_start(
                out=in_t[0][:, SPLIT0:],
                in_=xin[:, P * SPLIT0:P * c0].rearrange(
                    "x (p c) -> p (x c)", c=c0 - SPLIT0
                ),
            ).then_inc(in_done, 16)
            for i in range(1, NT):
                ci = ATTRS * TILES[i]
                sync.dma_start(
                    out=in_t[i][:, :],
                    in_=xin[:, P * CUM[i]:P * CUM[i + 1]].rearrange(
                        "x (p c) -> p (x c)", c=ci
                    ),
                ).then_inc(in_done, 16)
            for k in range(NT):
                ck = ATTRS * TILES[k]
                sync.wait_ge(dve_done, k + 1)
                sync.wait_ge(act_done, 3 * k + 3)  # exp lane written by ACT
                sync.dma_start(
                    out=yout[:, P * CUM[k]:P * CUM[k + 1]].rearrange(
                        "x (p c) -> p (x c)", c=ck
                    ),
                    in_=out_t[k][:, :],
                ).then_inc(out_done, 16)

        @block.scalar
        def _(scalar):
            # 1-element dummy triggers the ~1.3 us ACT_TABLE_LOAD under in0.
            nc.scalar.activation(t_t[0][:, 0:1], out_t[0][:, 0:1], AFT.Tanh)
            for i in range(NT):
                F = TILES[i]
                hs = TILE_HS[i]
                scalar.wait_ge(in_done, 16 * (i + 2) if i else 16)
                if i == 0:
                    scalar.wait_ge(const_done, 16)  # lnanc for the exp bias
                if i >= NSCR:
                    scalar.wait_ge(dve_done, i - NSCR + 1)  # t-scratch reuse
                in_r = in_t[i].rearrange("p (a j) -> p a j", a=ATTRS)
                t_r = t_t[i % NSCR].rearrange("p (a j) -> p a j", a=9)[:, :, :F]
                out_r = out_t[i].rearrange("p (a j) -> p a j", a=ATTRS)
                op_xyz = lambda: nc.scalar.activation(
                    t_r[:, 0:3, :], in_r[:, 0:3, :], AFT.Tanh, scale=0.5
                ).then_inc(act_done, 1)
                op_exp = lambda: nc.scalar.activation(
                    out_r[:, 3:4, :], in_r[:, 3:4, :], AFT.Exp,
                    bias=lnanc[:, hs:hs + 1],
                ).then_inc(act_done, 1)
                op_cls = lambda: nc.scalar.activation(
                    t_r[:, 3:9, :], in_r[:, 4:10, :], AFT.Tanh, scale=0.5
                ).then_inc(act_done, 1)
                if i == 0:
                    op_xyz(); op_exp()
                    scalar.wait_ge(in_done, 32)  # rows 4-9 of in0
                    op_cls()
                else:
                    op_cls(); op_xyz(); op_exp()

        @block.vector
        def _(vector):
            vector.wait_ge(const_done, 16)
            for i in range(NT):
                F = TILES[i]
                F1 = F // W
                hs = TILE_HS[i]
                g0 = TILE_OFF[i] // W
                t_r = t_t[i % NSCR].rearrange("p (a j) -> p a j", a=9)[:, :, :F]
                t_r4 = t_t[i % NSCR].rearrange(
                    "p (a j1 j0) -> p a j1 j0", a=9, j0=W
                )[:, :, :F1, :]
                out_r = out_t[i].rearrange("p (a j) -> p a j", a=ATTRS)
                out_r4 = out_t[i].rearrange(
                    "p (a j1 j0) -> p a j1 j0", a=ATTRS, j0=W
                )
                gx_bc = gxrow.unsqueeze(1).broadcast_to([P, F1, W])
                gy_bc = gysm[:, g0:g0 + F1].unsqueeze(2).broadcast_to([P, F1, W])
                op_big = lambda: nc.vector.tensor_scalar(
                    out_r[:, 4:10, :], t_r[:, 3:9, :], 0.5, 0.5, mult, add
                )
                op_x = lambda: nc.vector.scalar_tensor_tensor(
                    out_r4[:, 0], t_r4[:, 0], 2.0, gx_bc, mult, add
                )
                op_y = lambda: nc.vector.scalar_tensor_tensor(
                    out_r4[:, 1], t_r4[:, 1], 2.0, gy_bc, mult, add
                )
                op_z = lambda: nc.vector.tensor_scalar(
                    out_r[:, 2, :], t_r[:, 2, :], 2.0, gzb[:, hs:hs + 1], mult, add
                )
                if i == 0:
                    vector.wait_ge(act_done, 1)
                    op_x(); op_y(); op_z()
                    vector.wait_ge(act_done, 3)
                    op_big().then_inc(dve_done, 1)
                else:
                    vector.wait_ge(act_done, 3 * i + 1)
                    op_big()
                    vector.wait_ge(act_done, 3 * i + 2)
                    op_x(); op_y()
                    op_z().then_inc(dve_done, 1)

    return nc


def _host_constants():
    """Half-slab position s = p*R + off + j1*64 + j0:
      w = j0;  hgrid = 16*(p%4) + (off//64 + j1);  d = half*32 + p//4
    """
    p = np.arange(P)
    gxrow = np.broadcast_to(2.0 + 4.0 * np.arange(W), (P, W))
    rows = np.arange(NGY)  # off//64 + j1 over a full half-slab
    gysm = 2.0 + 4.0 * (16.0 * (p[:, None] % 4) + rows[None, :])
    base = np.concatenate([gxrow, gysm], axis=1)
    out = []
    for core in range(NCORES):
        gzb = np.empty((P, HS_PER_CORE), np.float32)
        lnanc = np.empty((P, HS_PER_CORE), np.float32)
        for k in range(HS_PER_CORE):
            hs_g = HS_PER_CORE * core + k
            slab, half = divmod(hs_g, 2)
            gzb[:, k] = 2.0 + 128.0 * half + 4.0 * (p // 4)
            lnanc[:, k] = np.log(ANCHOR_W[slab % A])
        out.append(np.concatenate([base, gzb, lnanc], axis=1).astype(np.float32))
    return out


def _run(inputs, trace=False):
    from concourse.bass_utils import run_bass_kernel_spmd

    x = np.asarray(inputs["input"])
    assert x.shape == (B, A * ATTRS, D, H, W), x.shape
    # -> f16 [hs_g, p, a, j(=R)] then concat per-tile [p, a, off:off+F]
    # column blocks so every load DMA is a straight contiguous memcpy.
    xh = np.ascontiguousarray(
        x.reshape(B * A, ATTRS, 2, P, R).transpose(0, 2, 3, 1, 4),
        dtype=np.float16,
    )  # [24, P, ATTRS, R] after merging slab+half
    xh = xh.reshape(B * A * 2, P, ATTRS, R)

    if "nc" not in _CACHE:
        _CACHE["nc"] = _build_nc()
        _CACHE["consts"] = _host_constants()
    nc = _CACHE["nc"]
    consts = _CACHE["consts"]

    in_maps = []
    for core in range(NCORES):
        pieces = []
        for i in range(NT):
            hs_g = HS_PER_CORE * core + TILE_HS[i]
            off, F = TILE_OFF[i], TILES[i]
            pieces.append(xh[hs_g, :, :, off:off + F].reshape(P, ATTRS * F))
        p0 = pieces[0].reshape(P, ATTRS, TILES[0])
        blocks = [p0[:, :4].reshape(1, -1), p0[:, 4:].reshape(1, -1)]
        blocks += [p.reshape(1, -1) for p in pieces[1:]]
        flat = np.concatenate(blocks, axis=1)
        in_maps.append({"xin": flat, "consts": consts[core]})

    res = run_bass_kernel_spmd(
        nc, in_maps, core_ids=list(range(NCORES)), trace=trace
    )
    _CACHE["last_exec_ns"] = res.exec_time_ns
    _CACHE["last_results"] = res

    # reassemble [hs_g, p, a, R] then -> [hs_g, p, j, a] -> [B, A*S, ATTRS]
    yh = np.empty((NCORES * HS_PER_CORE, P, ATTRS, R), np.float16)
    for core in range(NCORES):
        yc = res.results[core]["yout"]
        for i in range(NT):
            hs_g = HS_PER_CORE * core + TILE_HS[i]
            off, F = TILE_OFF[i], TILES[i]
            yh[hs_g, :, :, off:off + F] = yc.reshape(-1)[
                P * CUM[i]:P * CUM[i + 1]
            ].reshape(P, ATTRS, F)
    y = np.ascontiguousarray(yh.transpose(0, 1, 3, 2))
    return y.reshape(B, A * S, ATTRS).astype(np.float32)


def kernel(**inputs):
    return _run(inputs, trace=False)

